# revision 25
# baseline (speedup 1.0000x reference)
"""Distributed multi-head attention kernel for 8 TRN2 NeuronCores.

Problem: hidden[2,2048,1024] -> QKV proj (16 heads, hd=64) -> softmax
attention -> out proj. f32 I/O; bf16 projections, fp8e4 K/Q for the
score matmuls and fp8e4 probs/V for a DoubleRow ctx contraction.

Sharding: sequence-parallel. Flattened rows [4096, 1024] split into 8
chunks of 512 rows; cores 0-3 own batch 0, cores 4-7 batch 1. Each core
projects K^T and V for its own 512 rows and AllGathers both within its
4-core batch group. Collectives serialize per-rank on the ncfw and the
first one also absorbs the SPMD launch skew, so the four AGs are small
fp8 buffers (1MB gathered each) issued in consumption order
K0 -> V0 -> K1 -> V1, with all triggers ready before the skew window
closes: K^T runs k-outer right after the hT/wk loads, V n-outer next
(weights staged via the ACT engine's FIFO behind the hT/wk tails), Q
last. Outputs are disjoint row blocks concatenated on the host.

PE budget: score matmuls are row-tiled - head 2p contracts on array
rows 0-63, head 2p+1 on rows 64-127 (tile_position from the APs' base
partitions), so the two 64-dim contractions run concurrently. Scores
use single-bank [128,512] psum tiles (bufs=5) so the psum ring never
serializes the PE stream on softmax eviction. ctx uses fp8 DoubleRow
over two key tiles per matmul. A warmup burst at t=0 lifts the HAM
clock gate.

Engine budget: softmax exp per half-tile - ACT takes head-even halves
(plus every 4th head-odd), the rest are a one-op DVE Schraudolph: the
int8 value 1.4427*s + 55.65 IS the fp8e4m3 bit pattern of exp(s/8).
Denominators stash in f32, hop through DRAM, invert with one
reciprocal_approx_fast per pair; the flush is split in three stages
(DMA / DVE recip / PE selector+normalize) deferred so no engine
stream ever head-of-line blocks on it.
"""

import numpy as np
import ml_dtypes

B, S, D, H, HD = 2, 2048, 1024, 16, 64
N_CORES = 8
ROWS = (B * S) // N_CORES          # 512 query rows per core
GROUP = 4                          # cores per batch group
P = 128
KT = D // P                        # 8 contraction tiles over hidden dim
KEYT = S // P                      # 16 key tiles per batch
HA = HD + 1                        # head slot width in v_aug

_CACHE: dict = {}

bf16 = ml_dtypes.bfloat16
f8 = ml_dtypes.float8_e4m3


def _build_graph():
    import concourse.mybir as mybir
    import concourse.tile as tile
    from concourse import bacc
    from contextlib import ExitStack

    dt = mybir.dt
    F32, BF16, F8 = dt.float32, dt.bfloat16, dt.float8e4
    AF = mybir.ActivationFunctionType
    ALU = mybir.AluOpType
    DR = mybir.MatmulPerfMode.DoubleRow

    nc = bacc.Bacc("TRN2", target_bir_lowering=False, debug=False,
                   enable_asserts=False, num_devices=N_CORES)

    hT = nc.dram_tensor("hT", [D, ROWS], BF16, kind="ExternalInput").ap()
    wq = nc.dram_tensor("wq", [D, D], BF16, kind="ExternalInput").ap()
    wk = nc.dram_tensor("wk", [D, D], BF16, kind="ExternalInput").ap()
    wv = nc.dram_tensor("wv", [D, D], BF16, kind="ExternalInput").ap()
    wo = nc.dram_tensor("wo", [D, D], BF16, kind="ExternalInput").ap()
    bvr = nc.dram_tensor("bvr", [1, D], BF16, kind="ExternalInput").ap()
    bor = nc.dram_tensor("bor", [1, D], BF16, kind="ExternalInput").ap()
    bqk = nc.dram_tensor("bqk", [P, 2 * KT], F32, kind="ExternalInput").ap()
    sel = nc.dram_tensor("sel", [2, 2 * HD], BF16, kind="ExternalInput").ap()
    out = nc.dram_tensor("out", [ROWS, D], F32, kind="ExternalOutput").ap()

    RG = [[0, 1, 2, 3], [4, 5, 6, 7]]

    with tile.TileContext(nc) as tc, ExitStack() as top:
        dram = top.enter_context(tc.tile_pool(name="dram", bufs=1, space="DRAM"))
        pers = top.enter_context(tc.tile_pool(name="pers", bufs=1))
        attn = top.enter_context(tc.tile_pool(name="attn", bufs=1))

        HB = D // 2
        kb0 = dram.tile([HB, ROWS], F8)                 # kT bounce, heads 0-7
        kb1 = dram.tile([HB, ROWS], F8)                 # kT bounce, heads 8-15
        VBW = 8 * HA                                    # 8 head slots + ones
        vb0 = dram.tile([ROWS, VBW], F8)                # V bounce, heads 0-7
        vb1 = dram.tile([ROWS, VBW], F8)                # V bounce, heads 8-15
        KTg0 = dram.tile([GROUP * HB, ROWS], F8)        # gathered kT, heads 0-7
        KTg1 = dram.tile([GROUP * HB, ROWS], F8)        # gathered kT, heads 8-15
        Vg0 = dram.tile([GROUP * ROWS, VBW], F8)        # gathered V, heads 0-7
        Vg1 = dram.tile([GROUP * ROWS, VBW], F8)        # gathered V, heads 8-15
        dden = dram.tile([1, H * ROWS], F32)            # denominator hop

        dmy = dram.tile([1, 128], BF16)
        dmyg = dram.tile([GROUP, 128], BF16)

        ones_w = pers.tile([P, P], BF16)
        nc.vector.memset(ones_w[:], 1.0)
        # dummy collective at t=0: absorbs the ncfw startup + launch-skew
        # barrier while the projections run, so the real AllGather chain
        # starts ~1us after its trigger instead of 40-140us
        nc.sync.dma_start(dmy[0:1, :], ones_w[0:1, :])
        nc.gpsimd.collective_compute(
            "AllGather", mybir.AluOpType.bypass,
            replica_groups=[[0, 1, 2, 3], [4, 5, 6, 7]],
            ins=[dmy.opt()], outs=[dmyg.opt()])
        bqk_sb = pers.tile([P, 2 * KT], F32)
        nc.sync.dma_start(bqk_sb[:], bqk[:])
        sel_sb = pers.tile([2, 2 * HD], BF16)
        nc.sync.dma_start(sel_sb[:], sel[:])
        bvr_sb = pers.tile([1, D], BF16)
        nc.sync.dma_start(bvr_sb[:], bvr[:])
        bor_sb = pers.tile([1, D], BF16)
        nc.sync.dma_start(bor_sb[:], bor[:])
        bvb_sb = pers.tile([P, D], BF16)
        nc.gpsimd.partition_broadcast(bvb_sb[:], bvr_sb[:])
        bob_sb = pers.tile([P, D], BF16)
        nc.gpsimd.partition_broadcast(bob_sb[:], bor_sb[:])
        # per-head q^T slots (fp8): head 2j on partitions 0-63, head 2j+1
        # on 64-127; no zero padding - score matmuls are row-tiled
        qT_sb = pers.tile([P, H * ROWS], F8)

        kt_sb = attn.tile([P, 4 * KT * ROWS], F8)       # gathered K^T
        v_aug = attn.tile([P, KEYT * H * HA], F8)       # [V_h | 1] slots
        v4 = v_aug[:].rearrange("p (t h a) -> p t h a", h=H, a=HA)
        # pair-packed normalized ctx^T: head 2j on partitions 0-63 of pair
        # slot j, head 2j+1 on partitions 64-127 (odd heads arrive via a
        # cross-partition SBUF DMA from ctx_odd)
        ctx_pair = attn.tile([P, (H // 2) * ROWS], BF16)
        ctx_odd = attn.tile([HD, (H // 2) * ROWS], BF16)
        wo_sb = attn.tile([P, KT * D], BF16)

        # warmup burst: lift the HAM clock gate to 8/8 while the first
        # input DMAs are still in flight
        with ExitStack() as warm:
            ps_w = warm.enter_context(
                tc.tile_pool(name="ps_w", bufs=1, space="PSUM"))
            psw = ps_w.tile([P, P], F32, name="psw")
            for _ in range(60):
                nc.tensor.matmul(psw[:], ones_w[:], ones_w[:],
                                 start=True, stop=True)

        with ExitStack() as proj:
            wpool = proj.enter_context(tc.tile_pool(name="wpool", bufs=1))
            epool = proj.enter_context(tc.tile_pool(name="epool", bufs=4))
            ps_proj = proj.enter_context(
                tc.tile_pool(name="ps_proj", bufs=6, space="PSUM"))

            wk_sb = wpool.tile([P, KT * D], BF16)
            hT_sb = wpool.tile([P, KT * ROWS], BF16)
            wv_sb = wpool.tile([P, KT * D], BF16)
            wq_sb = wpool.tile([P, KT * D], BF16)
            # urgent input DMAs only: hT + wk feed the K projection;
            # single k-tile chunks so early tiles land early
            for kk in range(KT):
                nc.sync.dma_start(
                    hT_sb[:, kk * ROWS:(kk + 1) * ROWS],
                    hT[kk * P:(kk + 1) * P, :])
                nc.sync.dma_start(
                    wk_sb[:, kk * D:(kk + 1) * D],
                    wk[kk * P:(kk + 1) * P, :])

            def wload(dst, src, kk, n_k, eng):
                eng.dma_start(
                    dst[:, kk * D:(kk + n_k) * D]
                    .rearrange("p (k f) -> p k f", f=D),
                    src[kk * P:(kk + n_k) * P, :]
                    .rearrange("(k p) f -> p k f", p=P))

            # wv/wq/wo loads fire from the ACT engine's FIFO stream behind
            # tiny copies that read the last bytes of hT/wk, so they cannot
            # race the hT/wk loads for HBM bandwidth
            scrap = wpool.tile([1, 2], BF16)
            nc.scalar.activation(scrap[0:1, 0:1],
                                 hT_sb[0:1, KT * ROWS - 1:KT * ROWS],
                                 AF.Copy)
            nc.scalar.activation(scrap[0:1, 1:2],
                                 wk_sb[0:1, KT * D - 1:KT * D],
                                 AF.Copy)
            for kk in range(0, KT, 2):
                wload(wv_sb, wv, kk, 2, nc.scalar)
            for kk in (0, 4):
                wload(wq_sb, wq, kk, 4, nc.scalar)
            for kk in (0, 4):
                wload(wo_sb, wo, kk, 4, nc.scalar)

            # k^T projection, k-outer so matmuls start as soon as the
            # first hT/wk tiles land; bias bk folded into the fp8 eviction
            def kproj(ms, kbt):
                pss = {}
                for k in range(KT):
                    for m in ms:
                        if k == 0:
                            pss[m] = ps_proj.tile([P, ROWS], F32, name="ps")
                        nc.tensor.matmul(
                            pss[m][:],
                            wk_sb[:, k * D + m * P: k * D + (m + 1) * P],
                            hT_sb[:, k * ROWS:(k + 1) * ROWS],
                            start=(k == 0), stop=(k == KT - 1))
                for m in ms:
                    ev = epool.tile([P, ROWS], F8, name="ev")
                    nc.vector.tensor_scalar(
                        ev[:], pss[m][:], bqk_sb[:, KT + m: KT + m + 1], None,
                        ALU.add)
                    nc.sync.dma_start(
                        kbt[(m % 4) * P:(m % 4 + 1) * P, :], ev[:])

            kproj(range(0, 4), kb0)
            nc.gpsimd.collective_compute(
                "AllGather", mybir.AluOpType.bypass, replica_groups=RG,
                ins=[kb0.opt()], outs=[KTg0.opt()])
            kproj(range(4, 8), kb1)

            # q^T projection -> per-head fp8 slots (DVE eviction)
            def qproj(ms):
                for m in ms:
                    ps = ps_proj.tile([P, ROWS], F32, name="ps")
                    for k in range(KT):
                        nc.tensor.matmul(
                            ps[:],
                            wq_sb[:, k * D + m * P: k * D + (m + 1) * P],
                            hT_sb[:, k * ROWS:(k + 1) * ROWS],
                            start=(k == 0), stop=(k == KT - 1))
                    for hh in (0, 1):
                        h = 2 * m + hh
                        po = hh * HD
                        nc.vector.tensor_scalar(
                            qT_sb[po:po + HD, h * ROWS:(h + 1) * ROWS],
                            ps[po:po + HD, :],
                            bqk_sb[po:po + HD, m:m + 1], None,
                            ALU.add)

            # V projection for own rows, n-outer so the heads 0-7 half can
            # AllGather while heads 8-15 are still projecting; Q heads 0-7
            # project first so scores can start as soon as K0 lands
            def vproj(n):
                for mk in range(ROWS // P):
                    ps = ps_proj.tile([P, 512], F32, name="ps")
                    for k in range(KT):
                        nc.tensor.matmul(
                            ps[:],
                            hT_sb[:, k * ROWS + mk * P: k * ROWS + (mk + 1) * P],
                            wv_sb[:, k * D + n * 512: k * D + (n + 1) * 512],
                            start=(k == 0), stop=(k == KT - 1))
                    evv = epool.tile([P, VBW], F8, name="evv")
                    ev3 = evv[:].rearrange("p (h a) -> p h a", a=HA)
                    nc.vector.tensor_add(
                        ev3[:, :, 0:HD],
                        ps[:].rearrange("p (h d) -> p h d", d=HD),
                        bvb_sb[:, n * 512:(n + 1) * 512]
                        .rearrange("p (h d) -> p h d", d=HD))
                    nc.vector.memset(ev3[:, :, HD:HA], 1.0)
                    nc.sync.dma_start(
                        (vb0 if n == 0 else vb1)[mk * P:(mk + 1) * P, :],
                        evv[:])

            qproj(range(0, 4))
            vproj(0)
            nc.gpsimd.collective_compute(
                "AllGather", mybir.AluOpType.bypass, replica_groups=RG,
                ins=[vb0.opt()], outs=[Vg0.opt()])
            nc.gpsimd.collective_compute(
                "AllGather", mybir.AluOpType.bypass, replica_groups=RG,
                ins=[kb1.opt()], outs=[KTg1.opt()])
            vproj(1)
            nc.gpsimd.collective_compute(
                "AllGather", mybir.AluOpType.bypass, replica_groups=RG,
                ins=[vb1.opt()], outs=[Vg1.opt()])
            qproj(range(4, 8))

            # gathered K^T / V into SBUF, emitted in AG arrival order
            # (K0, V0, K1, V1) so no queued load head-of-line blocks a
            # ready one
            def kt_load(half, KTgh):
                for r in range(GROUP):
                    nc.sync.dma_start(
                        kt_sb[:, (r * KT + half * 4) * ROWS:
                              (r * KT + half * 4 + 4) * ROWS]
                        .rearrange("p (t f) -> p t f", f=ROWS),
                        KTgh[r * HB:(r + 1) * HB, :]
                        .rearrange("(t p) f -> p t f", p=P))

            def v_load(hhalf, Vgh):
                # four parallel DMAs - a single one walks the destination
                # partition-major and its 520B scattered source reads cap
                # one queue at ~35GB/s (~30us)
                for r in range(GROUP):
                    nc.sync.dma_start(
                        v_aug[:].rearrange("p (t x) -> p t x", x=H * HA)
                        [:, 4 * r:4 * (r + 1),
                         hhalf * VBW:(hhalf + 1) * VBW],
                        Vgh[r * 4 * P:(r + 1) * 4 * P, :]
                        .rearrange("(t p) f -> p t f", p=P))

            kt_load(0, KTg0)
            v_load(0, Vg0)
            kt_load(1, KTg1)
            v_load(1, Vg1)

        with ExitStack() as att:
            probs = att.enter_context(tc.tile_pool(name="probs", bufs=40))
            norm = att.enter_context(tc.tile_pool(name="norm", bufs=6))
            late = att.enter_context(tc.tile_pool(name="late", bufs=1))
            # per-head unnormalized ctx stash (f32): V rows 0..63, denom 64
            stash = late.tile([HA, H * ROWS], F32)

            with ExitStack() as attp:
                ps_s = attp.enter_context(
                    tc.tile_pool(name="ps_s", bufs=3, space="PSUM"))
                ps_ctx = attp.enter_context(
                    tc.tile_pool(name="ps_ctx", bufs=2, space="PSUM"))

                U = KEYT // 2
                LAG_U = 18
                NP = H // 2
                pend = {}
                psc = {}

                def emit_scores(p, u):
                    gidx = p * U + u
                    tiles = {}
                    for hh in (0, 1):
                        tiles[hh] = ps_s.tile([P, 2 * ROWS], F32,
                                              name="ps_sc")
                    for half in (0, 1):
                        t = 2 * u + half
                        r, m = divmod(t, KEYT // GROUP)
                        base = (r * KT + p) * ROWS
                        for hh in (0, 1):
                            h = 2 * p + hh
                            po = hh * HD
                            nc.tensor.matmul(
                                tiles[hh][:, half * ROWS:(half + 1) * ROWS],
                                kt_sb[po:po + HD,
                                      base + m * P: base + (m + 1) * P],
                                qT_sb[po:po + HD, h * ROWS:(h + 1) * ROWS],
                                start=True, stop=True)
                    for hh in (0, 1):
                        pt = probs.tile([P, 2 * ROWS], F8, name="pt")
                        if hh == 0 or gidx % 4 == 3:
                            nc.scalar.activation(pt[:], tiles[hh][:],
                                                 AF.Exp, scale=0.125)
                        else:
                            # one-op DVE Schraudolph: the int8 value
                            # 1.4427*s + 55.65 IS the fp8e4m3 bit
                            # pattern of exp(s/8)
                            nc.vector.tensor_scalar(
                                pt[:].bitcast(dt.int8), tiles[hh][:],
                                1.442695, 55.65,
                                ALU.mult, ALU.add)
                        pend[(2 * p + hh, u)] = pt

                def emit_ctx(p, u):
                    for hh in (0, 1):
                        h = 2 * p + hh
                        if u == 0:
                            psc[h] = ps_ctx.tile([HA, ROWS], F32, name="ps_c")
                        pt = pend.pop((h, u))
                        nc.tensor.matmul(
                            psc[h][:],
                            v4[:, 2 * u:2 * u + 2, h, :],
                            pt[:].rearrange("p (t f) -> p t f", f=ROWS),
                            start=(u == 0), stop=(u == U - 1),
                            perf_mode=DR)
                        if u == U - 1:
                            ps_c = psc.pop(h)
                            nc.vector.tensor_copy(
                                stash[:, h * ROWS:(h + 1) * ROWS], ps_c[:])

                rbs = {}

                def flush_dma(pq):
                    # denominator DRAM hop (sync queues only - no engine
                    # stream is blocked by it)
                    h0 = 2 * pq
                    nc.sync.dma_start(
                        dden[0:1, h0 * ROWS:(h0 + 2) * ROWS],
                        stash[HD:HD + 1, h0 * ROWS:(h0 + 2) * ROWS])
                    rns = []
                    for hh in (0, 1):
                        rn = norm.tile([1, ROWS], F32, name="rn")
                        nc.sync.dma_start(
                            rn[:],
                            dden[0:1, (h0 + hh) * ROWS:(h0 + hh + 1) * ROWS])
                        rns.append(rn)
                    rbs[pq] = rns

                def flush_recip(pq):
                    # deferred so the hop has landed by the time the DVE
                    # stream reaches the reciprocal
                    rns = rbs.pop(pq)
                    rrs = []
                    for hh in (0, 1):
                        rr = norm.tile([1, ROWS], F32, name="rr")
                        nc.vector.reciprocal_approx_fast(rr[:], rns[hh][:])
                        rrs.append(rr)
                    rbs[pq] = rrs

                def flush_norm(pq):
                    # reciprocal rows fan out on the otherwise-idle GPSIMD
                    # (partition broadcast) - no PE instruction anywhere in
                    # the flush path
                    rrs = rbs.pop(pq)
                    for hh in (0, 1):
                        h = 2 * pq + hh
                        rbb = norm.tile([HD, ROWS], F32, name="rbb")
                        nc.gpsimd.partition_broadcast(
                            rbb[:], rrs[hh][:])
                        if hh == 0:
                            dst = ctx_pair[0:HD, pq * ROWS:(pq + 1) * ROWS]
                        else:
                            dst = ctx_odd[:, pq * ROWS:(pq + 1) * ROWS]
                        nc.vector.tensor_mul(
                            dst, stash[0:HD, h * ROWS:(h + 1) * ROWS],
                            rbb[:])
                        if hh == 1:
                            nc.sync.dma_start(
                                ctx_pair[HD:P, pq * ROWS:(pq + 1) * ROWS],
                                ctx_odd[:, pq * ROWS:(pq + 1) * ROWS])

                fq = []
                for G in range(NP * U + LAG_U):
                    for due, fn, arg in [x for x in fq if x[0] <= G]:
                        fn(arg)
                        fq.remove((due, fn, arg))
                    if LAG_U <= G:
                        pc, uc = divmod(G - LAG_U, U)
                        emit_ctx(pc, uc)
                        if uc == U - 1:
                            flush_dma(pc)
                            fq.append((G + 2, flush_recip, pc))
                            fq.append((G + 5, flush_norm, pc))
                    if G < NP * U:
                        emit_scores(G // U, G % U)
                for due, fn, arg in sorted(fq):
                    if fn is flush_recip:
                        fn(arg)
                        fq.remove((due, fn, arg))

                # output projection shares the ps_s psum ring; the first
                # four (m,n) tiles pre-accumulate head pairs 0-6 between
                # the last flush stages so the PE stays warm through the
                # tail, and only the j=7 term waits on the final flush
                opool = att.enter_context(tc.tile_pool(name="opool", bufs=3))
                NJ = H // 2
                groups = [(m, n) for m in range(ROWS // P) for n in range(2)]

                def out_mm(ps, m, n, j, start, stop):
                    nc.tensor.matmul(
                        ps[:],
                        ctx_pair[:, j * ROWS + m * P: j * ROWS + (m + 1) * P],
                        wo_sb[:, j * D + n * 512: j * D + (n + 1) * 512],
                        start=start, stop=stop)

                def out_fin(ps, m, n):
                    ot = opool.tile([P, 512], F32, name="ot")
                    nc.vector.tensor_add(
                        ot[:], ps[:], bob_sb[:, n * 512:(n + 1) * 512])
                    nc.sync.dma_start(
                        out[m * P:(m + 1) * P, n * 512:(n + 1) * 512], ot[:])

                pre = {}
                for g in range(2):
                    m, n = groups[g]
                    ps = ps_s.tile([P, 512], F32, name="ps_sc")
                    for j in range(NJ - 1):
                        out_mm(ps, m, n, j, j == 0, False)
                    pre[g] = ps
                for due, fn, arg in sorted(fq):
                    fn(arg)
                for g in range(2):
                    m, n = groups[g]
                    ps = pre.pop(g)
                    out_mm(ps, m, n, NJ - 1, False, True)
                    out_fin(ps, m, n)
                for g in range(2, 8):
                    m, n = groups[g]
                    ps = ps_s.tile([P, 512], F32, name="ps_sc")
                    for j in range(NJ):
                        out_mm(ps, m, n, j, j == 0, j == NJ - 1)
                    out_fin(ps, m, n)

    nc.compile()
    return nc


def _prep_inputs(hidden_states, Wq, bq, Wk, bk, Wv, bv, Wo, bo):
    hs = np.asarray(hidden_states, np.float32).reshape(B * S, D)
    wq = np.asarray(Wq, np.float32).astype(bf16)
    wk = np.asarray(Wk, np.float32).astype(bf16)
    wv = np.asarray(Wv, np.float32).astype(bf16)
    wo = np.asarray(Wo, np.float32).astype(bf16)
    bvr = np.asarray(bv, np.float32).reshape(1, D).astype(bf16)
    bor = np.asarray(bo, np.float32).reshape(1, D).astype(bf16)
    sel = np.zeros((2, 2 * HD), np.float32)
    sel[0, 0:HD] = 1.0
    sel[1, HD:2 * HD] = 1.0
    sel = sel.astype(bf16)
    bqk = np.ascontiguousarray(np.concatenate(
        [np.asarray(bq, np.float32).reshape(KT, P).T,
         np.asarray(bk, np.float32).reshape(KT, P).T], 1).astype(np.float32))
    in_maps = []
    for c in range(N_CORES):
        hT = np.ascontiguousarray(
            hs[c * ROWS:(c + 1) * ROWS].T).astype(bf16)
        in_maps.append({"hT": hT, "wq": wq, "wk": wk,
                        "wv": wv, "wo": wo, "bvr": bvr, "bor": bor,
                        "bqk": bqk, "sel": sel})
    return in_maps


def _run(inputs, trace=False):
    from concourse import bass_utils
    if "nc" not in _CACHE:
        _CACHE["nc"] = _build_graph()
    nc = _CACHE["nc"]
    in_maps = _prep_inputs(**inputs)
    res = bass_utils.run_bass_kernel_spmd(
        nc, in_maps, core_ids=list(range(N_CORES)), trace=trace)
    full = np.concatenate([res.results[c]["out"] for c in range(N_CORES)],
                          axis=0).reshape(B, S, D).astype(np.float32)
    return full, res


def kernel(**inputs) -> np.ndarray:
    full, _ = _run(inputs, trace=False)
    return full
